# revision 21
# baseline (speedup 1.0000x reference)
"""Bass/Trainium2 kernel for nn_DreamAttention (dense transformer attention,
dead-softmax variant).

Math (per reference): q/k/v linear projections + RoPE, scores = q @ k^T /
sqrt(HD) (softmax computed but DISCARDED in the source), out = (scores @ v)
@ Wo^T.

Because no softmax is applied, attention is linear:
    (q @ k^T) @ v == q @ (k^T @ v)
so we compute the tiny per-head Gram matrix KV = k^T v  [HD, HD] instead of
the S x S score matrix (16x fewer FLOPs, no S x S materialization).

The q-side RoPE is folded into the attention matmul (RoPE is linear):
    attn_h = KV_h^T (cos*q_h) + KVp_h^T (sin* * q_h)
where KVp is KV with its partition halves swapped and sin* carries the
rotate-half signs. This lets the q projection emit feature-major tiles
directly (weight-stationary matmul), avoiding a transpose stage.

Sharding: data-parallel over tokens. 8 cores x 512 tokens (cores 0-3 hold
batch 0, cores 4-7 batch 1). Each core computes q/k/v for its tokens
(weights replicated), partial per-head KV over its tokens, an AllReduce of
the KV block within each 4-core batch group, then attn and the output
projection for its tokens. The scale 1/sqrt(HD) is folded into k's RoPE
tables on the host.

Changes vs the fp32r baseline (321us -> ~290us/rep):
  * bf16 datapath (weights, activations, matmul operands; fp32 PSUM
    accumulation; rel err 6.7e-3 vs the 2e-2 gate). Halves HBM traffic
    (fp32 weight streaming was ~180us/core, right at the PE floor).
  * k/v projections and the KV gram + AllReduce run per column-half
    (heads 0-7, then 8-15), so the first collective is in flight ~55us
    earlier and both hide under the q projection.
  * per-head KV matmuls use a 128-wide moving operand (bf16 has no
    fp32r >=256 free-dim constraint), halving wasted KV cycles.
  * tile pools persist across reps (kernel repetitions pipeline instead
    of paying a ~15us PE drain per boundary); x tiles double-buffered.
  * q projection runs in 4-tile PSUM rounds with scalar-engine eviction
    (own PSUM port) and the attention matmuls trail one round behind, so
    the PE never waits on the DVE bias+RoPE tail.

Measured on 8 trn2 cores: PE busy ~283.5us/rep at the chip's sustained
power-throttled clock (HAM 13/16 ~= 1.95 GHz; the 2.4 GHz roofline is
not reachable with all 8 cores under full matmul load), ~289.7us/rep
steady state = ~98% PE occupancy.
"""

import math
from contextlib import ExitStack

import ml_dtypes
import numpy as np

import concourse.mybir as mybir
import concourse.tile as tile
from concourse import bacc
from concourse import bass_utils

P = 128
HD = 128
F32 = mybir.dt.float32
BF = mybir.dt.bfloat16
BF_NP = ml_dtypes.bfloat16


def ts(i, size):
    return slice(i * size, (i + 1) * size)


def make_pools(tc, ctx):
    return {
        "sb": ctx.enter_context(tc.tile_pool(name="sb", bufs=1)),
        "ps": ctx.enter_context(tc.tile_pool(name="ps", bufs=8, space="PSUM")),
        "dram": ctx.enter_context(tc.tile_pool(name="dram", bufs=8,
                                               space="DRAM")),
    }


def emit_attn(tc, pools, io, t_core, d_model, replica_groups, rep=0):
    """Emit the per-core attention kernel.

    io: DRAM APs: xT [d_model, t_core] bf16; wqT/wkT/wvT/woT
    [d_model, d_model] bf16; bkb/bvb [128, d_model] f32 (broadcast biases);
    bqd [128, d_model/128] f32 (bq in feature-major per-tile columns);
    cosk/sinkf [t_core, HD] bf16 (token-major k tables, sign-folded +
    1/sqrt(HD) prescaled); cosqD/sinqD [128, t_core] bf16 (feature-major q
    tables, sinqD sign-folded); y [t_core, d_model] f32.
    """
    nc = tc.nc
    T_TILES = t_core // P
    DIN = d_model // P          # number of 128-wide feature tiles
    NH = d_model // HD          # heads
    N_CHUNK = 512
    PAIR = 2                    # chunks per W load / psum round
    HALF = PAIR * N_CHUNK       # columns per projection round
    N_SPLIT = d_model // HALF   # projection rounds (= collective splits)
    HS = NH // N_SPLIT          # heads per round
    QG = 4                      # q-proj feature tiles per psum round: small
                                # rounds trickle PSUM evictions + RoPE muls
                                # into the DVE early, so the attention phase
                                # never waits on the DVE tail
    h2 = HD // 2

    sb = pools["sb"]
    ps = pools["ps"]
    dram = pools["dram"]

    def psum(name, width, dtype=F32):
        return ps.tile([P, width], dtype, name=name, tag="ps", bufs=8)

    # ---- resident x^T tiles [din, t] (matmul operand for all projections).
    # Double-buffered across reps so the next rep's loads overlap this
    # rep's attn/output tail instead of stalling the PE at the boundary.
    xt_tiles = []
    for din in range(DIN):
        xt = sb.tile([P, t_core], BF, name=f"xt{din}", tag="xt",
                     bufs=2 * DIN)
        # sync queue: emitted ahead of this rep's Wk chunks and idle
        # through the previous rep's attn/o-proj tail, so the next rep's
        # x tiles are resident before the boundary (the scalar queue is
        # still draining Wo there)
        nc.sync.dma_start(xt[:], io["xT"][ts(din, P), :])
        xt_tiles.append(xt)

    # ---- k RoPE tables, resident across both halves ----
    ck_tiles, sk_tiles = [], []
    for t in range(T_TILES):
        ct = sb.tile([P, HD], BF, name=f"cosk{t}", tag="tab", bufs=4 * T_TILES)
        st = sb.tile([P, HD], BF, name=f"sink{t}", tag="tab", bufs=4 * T_TILES)
        nc.scalar.dma_start(ct[:], io["cosk"][ts(t, P), :])
        nc.scalar.dma_start(st[:], io["sinkf"][ts(t, P), :])
        ck_tiles.append(ct)
        sk_tiles.append(st)

    def wtile(name, width=HALF):
        return sb.tile([P, width], BF, name=name, tag="w", bufs=12)

    def project_half(wT_ap, bias_ap, out_tiles, half, dma_eng):
        """out[t, half cols] = x @ W^T + b for this round's 1024 columns."""
        psums = [psum(f"pp{i}", N_CHUNK) for i in range(T_TILES * PAIR)]
        for din in range(DIN):
            wt = wtile(f"w{din}")
            dma_eng.dma_start(wt[:], wT_ap[ts(din, P), ts(half, HALF)])
            for t in range(T_TILES):
                for p in range(PAIR):
                    nc.tensor.matmul(
                        psums[t * PAIR + p][:],
                        xt_tiles[din][:, ts(t, P)],
                        wt[:, ts(p, N_CHUNK)],
                        start=(din == 0),
                        stop=(din == DIN - 1),
                    )
        bt = sb.tile([P, HALF], F32, name=f"bias{half}", tag="bias", bufs=3)
        dma_eng.dma_start(bt[:], bias_ap[:, ts(half, HALF)])
        for t in range(T_TILES):
            for p in range(PAIR):
                nc.vector.tensor_add(
                    out_tiles[t][:, ts(half * PAIR + p, N_CHUNK)],
                    psums[t * PAIR + p][:],
                    bt[:, ts(p, N_CHUNK)],
                )

    def rope_half(tiles, half):
        """In-place RoPE on this half's heads of the token-major k tiles.

        out = x*cos + rot_half(x)*sin; sinf is sign-folded so
        rot_half(x)*sin == gather(x, +-64) * sinf elementwise.
        """
        def bc(ap2d, w):  # [128, w] -> [128, HS, w] broadcast view
            return ap2d.unsqueeze(1).broadcast_to([P, HS, w])

        for t in range(T_TILES):
            tmp = sb.tile([P, HALF], BF, name=f"ropetmp{t}", tag="rt", bufs=3)
            x3 = tiles[t][:, ts(half, HALF)].rearrange("p (h d) -> p h d",
                                                       d=HD)
            t3 = tmp[:].rearrange("p (h d) -> p h d", d=HD)
            ct, st = ck_tiles[t], sk_tiles[t]
            nc.vector.tensor_mul(t3[:, :, 0:h2], x3[:, :, h2:HD],
                                 bc(st[:, 0:h2], h2))
            nc.vector.tensor_mul(t3[:, :, h2:HD], x3[:, :, 0:h2],
                                 bc(st[:, h2:HD], h2))
            nc.vector.tensor_mul(x3, x3, bc(ct[:], HD))
            nc.vector.tensor_add(tiles[t][:, ts(half, HALF)],
                                 tiles[t][:, ts(half, HALF)], tmp[:])

    # ---- K/V projections, RoPE(k), per-head Gram KV[h] = k_h^T v_h and
    # its in-group AllReduce — all per column-half so the collective for
    # heads 0..7 is issued while heads 8..15 are still projecting ----
    k_tiles = [sb.tile([P, d_model], BF, name=f"k{t}", tag="kv", bufs=8)
               for t in range(T_TILES)]
    v_tiles = [sb.tile([P, d_model], BF, name=f"v{t}", tag="kv", bufs=8)
               for t in range(T_TILES)]
    kv_sb = sb.tile([P, d_model], BF, name="kvsb", tag="kvg", bufs=3)
    kv_red = sb.tile([P, d_model], BF, name="kvred", tag="kvg", bufs=3)
    kv_perm = sb.tile([P, d_model], BF, name="kvperm", tag="kvg", bufs=3)

    for s in range(N_SPLIT):
        project_half(io["wkT"], io["bkb"], k_tiles, s, nc.sync)
        project_half(io["wvT"], io["bvb"], v_tiles, s, nc.scalar)
        rope_half(k_tiles, s)
        for h in range(s * HS, (s + 1) * HS):
            kvp = psum(f"kvp{h}", HD)
            for t in range(T_TILES):
                nc.tensor.matmul(
                    kvp[:],
                    k_tiles[t][:, ts(h, HD)],
                    v_tiles[t][:, ts(h, HD)],
                    start=(t == 0),
                    stop=(t == T_TILES - 1),
                )
            nc.vector.tensor_copy(kv_sb[:, ts(h, HD)], kvp[:])
        kv_in = dram.tile([P, HS * HD], BF, name=f"kv_in{s}")
        kv_out = dram.tile([P, HS * HD], BF, name=f"kv_out{s}")
        nc.scalar.dma_start(kv_in[:], kv_sb[:, ts(s, HS * HD)])
        nc.gpsimd.collective_compute(
            "AllReduce",
            mybir.AluOpType.add,
            replica_groups=replica_groups,
            ins=[kv_in.opt()],
            outs=[kv_out.opt()],
        )
        nc.scalar.dma_start(kv_red[:, ts(s, HS * HD)], kv_out[:])
        # Partition-half-swapped copy for the folded q-side RoPE
        nc.scalar.dma_start(kv_perm[0:h2, ts(s, HS * HD)], kv_out[h2:HD, :])
        nc.scalar.dma_start(kv_perm[h2:HD, ts(s, HS * HD)], kv_out[0:h2, :])

    # ---- Q projection, feature-major: qD[dout, t] = W q-row blocks ----
    qcs = [None] * DIN
    bqd_sb = sb.tile([P, DIN], F32, name="bqd", tag="bqd", bufs=1)
    nc.sync.dma_start(bqd_sb[:], io["bqd"][:])
    cosq = sb.tile([P, t_core], BF, name="cosq", tag="qtab", bufs=2)
    sinq = sb.tile([P, t_core], BF, name="sinq", tag="qtab", bufs=2)
    nc.scalar.dma_start(cosq[:], io["cosqD"][:])
    nc.scalar.dma_start(sinq[:], io["sinqD"][:])
    # The attention matmuls trail the q projection by one group: group
    # g's heads hit the PE while group g+1 is still projecting, giving
    # the scalar-engine eviction + DVE RoPE muls a full group (~17us) of
    # slack. Only the last group's 4 heads run against a fresh DVE tail.
    attn_tiles = [None] * NH

    def emit_attn_heads(g):
        for h in range(g * QG, (g + 1) * QG):
            qc, qs = qcs[h]
            ap = psum(f"ap{h}", t_core)
            nc.tensor.matmul(ap[:], kv_red[:, ts(h, HD)], qc[:],
                             start=True, stop=False)
            nc.tensor.matmul(ap[:], kv_perm[:, ts(h, HD)], qs[:],
                             start=False, stop=True)
            asb = sb.tile([P, t_core], BF, name=f"asb{h}", tag="asb",
                          bufs=NH)
            nc.vector.tensor_copy(asb[:], ap[:])
            attn_tiles[h] = asb

    for g in range(DIN // QG):
        psums = [psum(f"qp{i}", t_core) for i in range(QG)]
        for din in range(DIN):
            wt = wtile(f"wq{din}", width=QG * P)
            nc.sync.dma_start(wt[:], io["wqT"][ts(din, P), ts(g, QG * P)])
            for j in range(QG):
                nc.tensor.matmul(
                    psums[j][:],
                    wt[:, ts(j, P)],
                    xt_tiles[din][:],
                    start=(din == 0),
                    stop=(din == DIN - 1),
                )
        for j in range(QG):
            dout = g * QG + j
            qd = sb.tile([P, t_core], BF, name=f"qd{dout}", tag="qd", bufs=4)
            nc.scalar.activation(qd[:], psums[j][:],
                                 mybir.ActivationFunctionType.Identity,
                                 bias=bqd_sb[:, dout:dout + 1])
            qc = sb.tile([P, t_core], BF, name=f"qc{dout}", tag="qcs",
                         bufs=4 * QG)
            nc.vector.tensor_mul(qc[:], qd[:], cosq[:])
            qs = sb.tile([P, t_core], BF, name=f"qs{dout}", tag="qcs",
                         bufs=4 * QG)
            nc.vector.tensor_mul(qs[:], qd[:], sinq[:])
            qcs[dout] = (qc, qs)
        # attn_h[d2, t] = KV_h^T (cos*q_h) + KVp_h^T (sin* q_h), one
        # group behind the projection
        if g >= 1:
            emit_attn_heads(g - 1)
    emit_attn_heads(DIN // QG - 1)

    # ---- Output projection: y[t, dout] = attn @ Wo^T, token-major ----
    for half in range(N_SPLIT):
        psums = [psum(f"op{i}", N_CHUNK) for i in range(T_TILES * PAIR)]
        for dmid in range(DIN):
            wt = wtile(f"wo{dmid}")
            nc.scalar.dma_start(wt[:], io["woT"][ts(dmid, P), ts(half, HALF)])
            for t in range(T_TILES):
                for p in range(PAIR):
                    nc.tensor.matmul(
                        psums[t * PAIR + p][:],
                        attn_tiles[dmid][:, ts(t, P)],
                        wt[:, ts(p, N_CHUNK)],
                        start=(dmid == 0),
                        stop=(dmid == DIN - 1),
                    )
        for t in range(T_TILES):
            for p in range(PAIR):
                chunk = half * PAIR + p
                osb = sb.tile([P, N_CHUNK], F32, name=f"osb{chunk}_{t}",
                              tag="osb", bufs=4)
                nc.vector.tensor_copy(osb[:], psums[t * PAIR + p][:])
                nc.gpsimd.dma_start(io["y"][ts(t, P), ts(chunk, N_CHUNK)],
                                    osb[:])


def build_nc(t_core, d_model, num_devices, replica_groups, reps=1):
    nc = bacc.Bacc("TRN2", target_bir_lowering=False, debug=False,
                   num_devices=num_devices)
    io = {}
    io["xT"] = nc.dram_tensor("xT", [d_model, t_core], BF,
                              kind="ExternalInput").ap()
    for nm in ("wqT", "wkT", "wvT", "woT"):
        io[nm] = nc.dram_tensor(nm, [d_model, d_model], BF,
                                kind="ExternalInput").ap()
    for nm in ("bkb", "bvb"):
        io[nm] = nc.dram_tensor(nm, [P, d_model], F32,
                                kind="ExternalInput").ap()
    io["bqd"] = nc.dram_tensor("bqd", [P, d_model // P], F32,
                               kind="ExternalInput").ap()
    for nm in ("cosk", "sinkf"):
        io[nm] = nc.dram_tensor(nm, [t_core, HD], BF,
                                kind="ExternalInput").ap()
    for nm in ("cosqD", "sinqD"):
        io[nm] = nc.dram_tensor(nm, [P, t_core], BF,
                                kind="ExternalInput").ap()
    io["y"] = nc.dram_tensor("y", [t_core, d_model], F32,
                             kind="ExternalOutput").ap()

    with tile.TileContext(nc) as tc:
        with ExitStack() as ctx:
            pools = make_pools(tc, ctx)
            for r in range(reps):
                emit_attn(tc, pools, io, t_core, d_model, replica_groups,
                          rep=r)
    nc.compile()
    return nc


# ---------------- host side ----------------

B, S, D = 2, 2048, 2048
NH_FULL = 16
MAX_POS = 4096
ROPE_THETA = 10000.0
N_CORES = 8
T_CORE = B * S // N_CORES

_cache = {}


def _rope_tables():
    inv_freq = (np.float32(1.0) /
                np.power(np.float32(ROPE_THETA),
                         np.arange(0, HD, 2, dtype=np.float32) /
                         np.float32(HD))).astype(np.float32)
    t = np.arange(MAX_POS, dtype=np.float32)
    freqs = np.outer(t, inv_freq).astype(np.float32)
    emb = np.concatenate((freqs, freqs), axis=-1)
    return np.cos(emb).astype(np.float32), np.sin(emb).astype(np.float32)


def _get_nc():
    if "nc" not in _cache:
        _cache["nc"] = build_nc(T_CORE, D, N_CORES,
                                [[0, 1, 2, 3], [4, 5, 6, 7]])
    return _cache["nc"]


def _host_inputs(hidden_states, position_ids, Wq, bq, Wk, bk, Wv, bv, Wo):
    x = np.asarray(hidden_states, dtype=np.float32).reshape(B * S, D)
    pos = np.asarray(position_ids).astype(np.int64).reshape(B * S)

    cos_t, sin_t = _rope_tables()
    cos = cos_t[pos]            # [B*S, HD]
    sin = sin_t[pos]
    # token-major k tables: sign-folded sin + 1/sqrt(HD) fold
    sinf = sin.copy()
    sinf[:, : HD // 2] *= np.float32(-1.0)
    scale = np.float32(1.0 / math.sqrt(HD))
    # feature-major q tables: sin* = +sin (i<64), -sin (i>=64)
    sinq = sin.copy()
    sinq[:, HD // 2:] *= np.float32(-1.0)

    wqT = np.ascontiguousarray(np.asarray(Wq, np.float32).T.astype(BF_NP))
    wkT = np.ascontiguousarray(np.asarray(Wk, np.float32).T.astype(BF_NP))
    wvT = np.ascontiguousarray(np.asarray(Wv, np.float32).T.astype(BF_NP))
    woT = np.ascontiguousarray(np.asarray(Wo, np.float32).T.astype(BF_NP))
    bkb = np.ascontiguousarray(
        np.broadcast_to(np.asarray(bk, np.float32), (P, D)))
    bvb = np.ascontiguousarray(
        np.broadcast_to(np.asarray(bv, np.float32), (P, D)))
    bqd = np.ascontiguousarray(np.asarray(bq, np.float32).reshape(D // P, P).T)

    in_maps = []
    for c in range(N_CORES):
        sl = slice(c * T_CORE, (c + 1) * T_CORE)
        in_maps.append({
            "xT": np.ascontiguousarray(x[sl].T.astype(BF_NP)),
            "wqT": wqT, "wkT": wkT, "wvT": wvT, "woT": woT,
            "bkb": bkb, "bvb": bvb, "bqd": bqd,
            "cosk": np.ascontiguousarray((cos[sl] * scale).astype(BF_NP)),
            "sinkf": np.ascontiguousarray((sinf[sl] * scale).astype(BF_NP)),
            "cosqD": np.ascontiguousarray(cos[sl].T.astype(BF_NP)),
            "sinqD": np.ascontiguousarray(sinq[sl].T.astype(BF_NP)),
        })
    return in_maps


def kernel(hidden_states, position_ids, Wq, bq, Wk, bk, Wv, bv, Wo):
    in_maps = _host_inputs(hidden_states, position_ids,
                           Wq, bq, Wk, bk, Wv, bv, Wo)
    nc = _get_nc()
    last_err = None
    for attempt in range(3):
        try:
            res = bass_utils.run_bass_kernel_spmd(
                nc, in_maps, core_ids=list(range(N_CORES)))
            break
        except Exception as e:  # transient axon/device states clear on retry
            last_err = e
            import time
            time.sleep(15 * (attempt + 1))
    else:
        raise last_err
    out = np.concatenate([res.results[c]["y"] for c in range(N_CORES)], axis=0)
    return out.reshape(B, S, D)


# revision 23
# speedup vs baseline: 1.0104x; 1.0104x over previous
"""Bass/Trainium2 kernel for nn_DreamAttention (dense transformer attention,
dead-softmax variant).

Math (per reference): q/k/v linear projections + RoPE, scores = q @ k^T /
sqrt(HD) (softmax computed but DISCARDED in the source), out = (scores @ v)
@ Wo^T.

Because no softmax is applied, attention is linear:
    (q @ k^T) @ v == q @ (k^T @ v)
so we compute the tiny per-head Gram matrix KV = k^T v  [HD, HD] instead of
the S x S score matrix (16x fewer FLOPs, no S x S materialization).

The q-side RoPE is folded into the attention matmul (RoPE is linear):
    attn_h = KV_h^T (cos*q_h) + KVp_h^T (sin* * q_h)
where KVp is KV with its partition halves swapped and sin* carries the
rotate-half signs. This lets the q projection emit feature-major tiles
directly (weight-stationary matmul), avoiding a transpose stage.

Sharding: data-parallel over tokens. 8 cores x 512 tokens (cores 0-3 hold
batch 0, cores 4-7 batch 1). Each core computes q/k/v for its tokens
(weights replicated), partial per-head KV over its tokens, an AllReduce of
the KV block within each 4-core batch group, then attn and the output
projection for its tokens. The scale 1/sqrt(HD) is folded into k's RoPE
tables on the host.

Changes vs the fp32r baseline (321us -> ~290us/rep):
  * bf16 datapath (weights, activations, matmul operands; fp32 PSUM
    accumulation; rel err 6.7e-3 vs the 2e-2 gate). Halves HBM traffic
    (fp32 weight streaming was ~180us/core, right at the PE floor).
  * k/v projections and the KV gram + AllReduce run per column-half
    (heads 0-7, then 8-15), so the first collective is in flight ~55us
    earlier and both hide under the q projection.
  * per-head KV matmuls use a 128-wide moving operand (bf16 has no
    fp32r >=256 free-dim constraint), halving wasted KV cycles.
  * tile pools persist across reps (kernel repetitions pipeline instead
    of paying a ~15us PE drain per boundary); x tiles double-buffered.
  * q projection runs in 4-tile PSUM rounds with scalar-engine eviction
    (own PSUM port) and the attention matmuls trail one round behind, so
    the PE never waits on the DVE bias+RoPE tail.

Measured on 8 trn2 cores: PE busy ~283.5us/rep at the chip's sustained
power-throttled clock (HAM 13/16 ~= 1.95 GHz; the 2.4 GHz roofline is
not reachable with all 8 cores under full matmul load), ~289.7us/rep
steady state = ~98% PE occupancy.
"""

import math
from contextlib import ExitStack

import ml_dtypes
import numpy as np

import concourse.mybir as mybir
import concourse.tile as tile
from concourse import bacc
from concourse import bass_utils

P = 128
HD = 128
F32 = mybir.dt.float32
BF = mybir.dt.bfloat16
BF_NP = ml_dtypes.bfloat16


def ts(i, size):
    return slice(i * size, (i + 1) * size)


def make_pools(tc, ctx):
    return {
        "sb": ctx.enter_context(tc.tile_pool(name="sb", bufs=1)),
        "ps": ctx.enter_context(tc.tile_pool(name="ps", bufs=8, space="PSUM")),
        "dram": ctx.enter_context(tc.tile_pool(name="dram", bufs=8,
                                               space="DRAM")),
    }


def emit_attn(tc, pools, io, t_core, d_model, replica_groups, rep=0):
    """Emit the per-core attention kernel.

    io: DRAM APs: xT [d_model, t_core] bf16; wqT/wkT/wvT/woT
    [d_model, d_model] bf16; bkb/bvb [128, d_model] f32 (broadcast biases);
    bqd [128, d_model/128] f32 (bq in feature-major per-tile columns);
    cosk/sinkf [t_core, HD] bf16 (token-major k tables, sign-folded +
    1/sqrt(HD) prescaled); cosqD/sinqD [128, t_core] bf16 (feature-major q
    tables, sinqD sign-folded); y [t_core, d_model] f32.
    """
    nc = tc.nc
    T_TILES = t_core // P
    DIN = d_model // P          # number of 128-wide feature tiles
    NH = d_model // HD          # heads
    N_CHUNK = 512
    PAIR = 2                    # chunks per W load / psum round
    HALF = PAIR * N_CHUNK       # columns per projection round
    N_SPLIT = d_model // HALF   # projection rounds (= collective splits)
    HS = NH // N_SPLIT          # heads per round
    QG = 4                      # q-proj feature tiles per psum round: small
                                # rounds trickle PSUM evictions + RoPE muls
                                # into the DVE early, so the attention phase
                                # never waits on the DVE tail
    h2 = HD // 2

    sb = pools["sb"]
    ps = pools["ps"]
    dram = pools["dram"]

    def psum(name, width, dtype=F32):
        return ps.tile([P, width], dtype, name=name, tag="ps", bufs=8)

    # ---- resident x^T tiles [din, t] (matmul operand for all projections).
    # Double-buffered across reps so the next rep's loads overlap this
    # rep's attn/output tail instead of stalling the PE at the boundary.
    # Loads split across both HWDGE queues: all on scalar they queue
    # behind the previous rep's Wo burst (boundary stall); all on sync
    # they starve the K-projection W chunks. Half on each keeps both
    # bursts short.
    xt_tiles = []
    for din in range(DIN):
        xt = sb.tile([P, t_core], BF, name=f"xt{din}", tag="xt",
                     bufs=2 * DIN)
        eng = nc.sync if din % 2 == 0 else nc.scalar
        eng.dma_start(xt[:], io["xT"][ts(din, P), :])
        xt_tiles.append(xt)

    # ---- k RoPE tables, resident across both halves ----
    ck_tiles, sk_tiles = [], []
    for t in range(T_TILES):
        ct = sb.tile([P, HD], BF, name=f"cosk{t}", tag="tab", bufs=4 * T_TILES)
        st = sb.tile([P, HD], BF, name=f"sink{t}", tag="tab", bufs=4 * T_TILES)
        nc.scalar.dma_start(ct[:], io["cosk"][ts(t, P), :])
        nc.scalar.dma_start(st[:], io["sinkf"][ts(t, P), :])
        ck_tiles.append(ct)
        sk_tiles.append(st)

    def wtile(name, width=HALF):
        return sb.tile([P, width], BF, name=name, tag="w", bufs=12)

    def project_half(wT_ap, bias_ap, out_tiles, half, dma_eng):
        """out[t, half cols] = x @ W^T + b for this round's 1024 columns."""
        psums = [psum(f"pp{i}", N_CHUNK) for i in range(T_TILES * PAIR)]
        for din in range(DIN):
            wt = wtile(f"w{din}")
            dma_eng.dma_start(wt[:], wT_ap[ts(din, P), ts(half, HALF)])
            for t in range(T_TILES):
                for p in range(PAIR):
                    nc.tensor.matmul(
                        psums[t * PAIR + p][:],
                        xt_tiles[din][:, ts(t, P)],
                        wt[:, ts(p, N_CHUNK)],
                        start=(din == 0),
                        stop=(din == DIN - 1),
                    )
        bt = sb.tile([P, HALF], F32, name=f"bias{half}", tag="bias", bufs=3)
        dma_eng.dma_start(bt[:], bias_ap[:, ts(half, HALF)])
        for t in range(T_TILES):
            for p in range(PAIR):
                nc.vector.tensor_add(
                    out_tiles[t][:, ts(half * PAIR + p, N_CHUNK)],
                    psums[t * PAIR + p][:],
                    bt[:, ts(p, N_CHUNK)],
                )

    def rope_half(tiles, half):
        """In-place RoPE on this half's heads of the token-major k tiles.

        out = x*cos + rot_half(x)*sin; sinf is sign-folded so
        rot_half(x)*sin == gather(x, +-64) * sinf elementwise.
        """
        def bc(ap2d, w):  # [128, w] -> [128, HS, w] broadcast view
            return ap2d.unsqueeze(1).broadcast_to([P, HS, w])

        for t in range(T_TILES):
            tmp = sb.tile([P, HALF], BF, name=f"ropetmp{t}", tag="rt", bufs=3)
            x3 = tiles[t][:, ts(half, HALF)].rearrange("p (h d) -> p h d",
                                                       d=HD)
            t3 = tmp[:].rearrange("p (h d) -> p h d", d=HD)
            ct, st = ck_tiles[t], sk_tiles[t]
            nc.vector.tensor_mul(t3[:, :, 0:h2], x3[:, :, h2:HD],
                                 bc(st[:, 0:h2], h2))
            nc.vector.tensor_mul(t3[:, :, h2:HD], x3[:, :, 0:h2],
                                 bc(st[:, h2:HD], h2))
            nc.vector.tensor_mul(x3, x3, bc(ct[:], HD))
            nc.vector.tensor_add(tiles[t][:, ts(half, HALF)],
                                 tiles[t][:, ts(half, HALF)], tmp[:])

    # ---- K/V projections, RoPE(k), per-head Gram KV[h] = k_h^T v_h and
    # its in-group AllReduce — all per column-half so the collective for
    # heads 0..7 is issued while heads 8..15 are still projecting ----
    k_tiles = [sb.tile([P, d_model], BF, name=f"k{t}", tag="kv", bufs=8)
               for t in range(T_TILES)]
    v_tiles = [sb.tile([P, d_model], BF, name=f"v{t}", tag="kv", bufs=8)
               for t in range(T_TILES)]
    kv_sb = sb.tile([P, d_model], BF, name="kvsb", tag="kvg", bufs=3)
    kv_red = sb.tile([P, d_model], BF, name="kvred", tag="kvg", bufs=3)
    kv_perm = sb.tile([P, d_model], BF, name="kvperm", tag="kvg", bufs=3)

    for s in range(N_SPLIT):
        project_half(io["wkT"], io["bkb"], k_tiles, s, nc.sync)
        project_half(io["wvT"], io["bvb"], v_tiles, s, nc.scalar)
        rope_half(k_tiles, s)
        for h in range(s * HS, (s + 1) * HS):
            kvp = psum(f"kvp{h}", HD)
            for t in range(T_TILES):
                nc.tensor.matmul(
                    kvp[:],
                    k_tiles[t][:, ts(h, HD)],
                    v_tiles[t][:, ts(h, HD)],
                    start=(t == 0),
                    stop=(t == T_TILES - 1),
                )
            nc.vector.tensor_copy(kv_sb[:, ts(h, HD)], kvp[:])
        kv_in = dram.tile([P, HS * HD], BF, name=f"kv_in{s}")
        kv_out = dram.tile([P, HS * HD], BF, name=f"kv_out{s}")
        nc.scalar.dma_start(kv_in[:], kv_sb[:, ts(s, HS * HD)])
        nc.gpsimd.collective_compute(
            "AllReduce",
            mybir.AluOpType.add,
            replica_groups=replica_groups,
            ins=[kv_in.opt()],
            outs=[kv_out.opt()],
        )
        nc.sync.dma_start(kv_red[:, ts(s, HS * HD)], kv_out[:])
        # Partition-half-swapped copy for the folded q-side RoPE
        nc.sync.dma_start(kv_perm[0:h2, ts(s, HS * HD)], kv_out[h2:HD, :])
        nc.sync.dma_start(kv_perm[h2:HD, ts(s, HS * HD)], kv_out[0:h2, :])

    # ---- Q projection, feature-major: qD[dout, t] = W q-row blocks ----
    qcs = [None] * DIN
    bqd_sb = sb.tile([P, DIN], F32, name="bqd", tag="bqd", bufs=1)
    nc.sync.dma_start(bqd_sb[:], io["bqd"][:])
    cosq = sb.tile([P, t_core], BF, name="cosq", tag="qtab", bufs=2)
    sinq = sb.tile([P, t_core], BF, name="sinq", tag="qtab", bufs=2)
    nc.scalar.dma_start(cosq[:], io["cosqD"][:])
    nc.scalar.dma_start(sinq[:], io["sinqD"][:])
    # The attention matmuls trail the q projection by one group: group
    # g's heads hit the PE while group g+1 is still projecting, giving
    # the scalar-engine eviction + DVE RoPE muls a full group (~17us) of
    # slack. Only the last group's 4 heads run against a fresh DVE tail.
    attn_tiles = [None] * NH

    def emit_attn_heads(g):
        for h in range(g * QG, (g + 1) * QG):
            qc, qs = qcs[h]
            ap = psum(f"ap{h}", t_core)
            nc.tensor.matmul(ap[:], kv_red[:, ts(h, HD)], qc[:],
                             start=True, stop=False)
            nc.tensor.matmul(ap[:], kv_perm[:, ts(h, HD)], qs[:],
                             start=False, stop=True)
            asb = sb.tile([P, t_core], BF, name=f"asb{h}", tag="asb",
                          bufs=NH)
            nc.vector.tensor_copy(asb[:], ap[:])
            attn_tiles[h] = asb

    for g in range(DIN // QG):
        psums = [psum(f"qp{i}", t_core) for i in range(QG)]
        for din in range(DIN):
            wt = wtile(f"wq{din}", width=QG * P)
            nc.sync.dma_start(wt[:], io["wqT"][ts(din, P), ts(g, QG * P)])
            for j in range(QG):
                nc.tensor.matmul(
                    psums[j][:],
                    wt[:, ts(j, P)],
                    xt_tiles[din][:],
                    start=(din == 0),
                    stop=(din == DIN - 1),
                )
        for j in range(QG):
            dout = g * QG + j
            qd = sb.tile([P, t_core], BF, name=f"qd{dout}", tag="qd", bufs=4)
            nc.scalar.activation(qd[:], psums[j][:],
                                 mybir.ActivationFunctionType.Identity,
                                 bias=bqd_sb[:, dout:dout + 1])
            qc = sb.tile([P, t_core], BF, name=f"qc{dout}", tag="qcs",
                         bufs=4 * QG)
            nc.vector.tensor_mul(qc[:], qd[:], cosq[:])
            qs = sb.tile([P, t_core], BF, name=f"qs{dout}", tag="qcs",
                         bufs=4 * QG)
            nc.vector.tensor_mul(qs[:], qd[:], sinq[:])
            qcs[dout] = (qc, qs)
        # attn_h[d2, t] = KV_h^T (cos*q_h) + KVp_h^T (sin* q_h), one
        # group behind the projection
        if g >= 1:
            emit_attn_heads(g - 1)
    emit_attn_heads(DIN // QG - 1)

    # ---- Output projection: y[t, dout] = attn @ Wo^T, token-major ----
    for half in range(N_SPLIT):
        psums = [psum(f"op{i}", N_CHUNK) for i in range(T_TILES * PAIR)]
        for dmid in range(DIN):
            wt = wtile(f"wo{dmid}")
            nc.scalar.dma_start(wt[:], io["woT"][ts(dmid, P), ts(half, HALF)])
            for t in range(T_TILES):
                for p in range(PAIR):
                    nc.tensor.matmul(
                        psums[t * PAIR + p][:],
                        attn_tiles[dmid][:, ts(t, P)],
                        wt[:, ts(p, N_CHUNK)],
                        start=(dmid == 0),
                        stop=(dmid == DIN - 1),
                    )
        for t in range(T_TILES):
            for p in range(PAIR):
                chunk = half * PAIR + p
                osb = sb.tile([P, N_CHUNK], F32, name=f"osb{chunk}_{t}",
                              tag="osb", bufs=4)
                nc.vector.tensor_copy(osb[:], psums[t * PAIR + p][:])
                nc.gpsimd.dma_start(io["y"][ts(t, P), ts(chunk, N_CHUNK)],
                                    osb[:])


def build_nc(t_core, d_model, num_devices, replica_groups, reps=1):
    nc = bacc.Bacc("TRN2", target_bir_lowering=False, debug=False,
                   num_devices=num_devices)
    io = {}
    io["xT"] = nc.dram_tensor("xT", [d_model, t_core], BF,
                              kind="ExternalInput").ap()
    for nm in ("wqT", "wkT", "wvT", "woT"):
        io[nm] = nc.dram_tensor(nm, [d_model, d_model], BF,
                                kind="ExternalInput").ap()
    for nm in ("bkb", "bvb"):
        io[nm] = nc.dram_tensor(nm, [P, d_model], F32,
                                kind="ExternalInput").ap()
    io["bqd"] = nc.dram_tensor("bqd", [P, d_model // P], F32,
                               kind="ExternalInput").ap()
    for nm in ("cosk", "sinkf"):
        io[nm] = nc.dram_tensor(nm, [t_core, HD], BF,
                                kind="ExternalInput").ap()
    for nm in ("cosqD", "sinqD"):
        io[nm] = nc.dram_tensor(nm, [P, t_core], BF,
                                kind="ExternalInput").ap()
    io["y"] = nc.dram_tensor("y", [t_core, d_model], F32,
                             kind="ExternalOutput").ap()

    with tile.TileContext(nc) as tc:
        with ExitStack() as ctx:
            pools = make_pools(tc, ctx)
            for r in range(reps):
                emit_attn(tc, pools, io, t_core, d_model, replica_groups,
                          rep=r)
    nc.compile()
    return nc


# ---------------- host side ----------------

B, S, D = 2, 2048, 2048
NH_FULL = 16
MAX_POS = 4096
ROPE_THETA = 10000.0
N_CORES = 8
T_CORE = B * S // N_CORES

_cache = {}


def _rope_tables():
    inv_freq = (np.float32(1.0) /
                np.power(np.float32(ROPE_THETA),
                         np.arange(0, HD, 2, dtype=np.float32) /
                         np.float32(HD))).astype(np.float32)
    t = np.arange(MAX_POS, dtype=np.float32)
    freqs = np.outer(t, inv_freq).astype(np.float32)
    emb = np.concatenate((freqs, freqs), axis=-1)
    return np.cos(emb).astype(np.float32), np.sin(emb).astype(np.float32)


def _get_nc():
    if "nc" not in _cache:
        _cache["nc"] = build_nc(T_CORE, D, N_CORES,
                                [[0, 1, 2, 3], [4, 5, 6, 7]])
    return _cache["nc"]


def _host_inputs(hidden_states, position_ids, Wq, bq, Wk, bk, Wv, bv, Wo):
    x = np.asarray(hidden_states, dtype=np.float32).reshape(B * S, D)
    pos = np.asarray(position_ids).astype(np.int64).reshape(B * S)

    cos_t, sin_t = _rope_tables()
    cos = cos_t[pos]            # [B*S, HD]
    sin = sin_t[pos]
    # token-major k tables: sign-folded sin + 1/sqrt(HD) fold
    sinf = sin.copy()
    sinf[:, : HD // 2] *= np.float32(-1.0)
    scale = np.float32(1.0 / math.sqrt(HD))
    # feature-major q tables: sin* = +sin (i<64), -sin (i>=64)
    sinq = sin.copy()
    sinq[:, HD // 2:] *= np.float32(-1.0)

    wqT = np.ascontiguousarray(np.asarray(Wq, np.float32).T.astype(BF_NP))
    wkT = np.ascontiguousarray(np.asarray(Wk, np.float32).T.astype(BF_NP))
    wvT = np.ascontiguousarray(np.asarray(Wv, np.float32).T.astype(BF_NP))
    woT = np.ascontiguousarray(np.asarray(Wo, np.float32).T.astype(BF_NP))
    bkb = np.ascontiguousarray(
        np.broadcast_to(np.asarray(bk, np.float32), (P, D)))
    bvb = np.ascontiguousarray(
        np.broadcast_to(np.asarray(bv, np.float32), (P, D)))
    bqd = np.ascontiguousarray(np.asarray(bq, np.float32).reshape(D // P, P).T)

    in_maps = []
    for c in range(N_CORES):
        sl = slice(c * T_CORE, (c + 1) * T_CORE)
        in_maps.append({
            "xT": np.ascontiguousarray(x[sl].T.astype(BF_NP)),
            "wqT": wqT, "wkT": wkT, "wvT": wvT, "woT": woT,
            "bkb": bkb, "bvb": bvb, "bqd": bqd,
            "cosk": np.ascontiguousarray((cos[sl] * scale).astype(BF_NP)),
            "sinkf": np.ascontiguousarray((sinf[sl] * scale).astype(BF_NP)),
            "cosqD": np.ascontiguousarray(cos[sl].T.astype(BF_NP)),
            "sinqD": np.ascontiguousarray(sinq[sl].T.astype(BF_NP)),
        })
    return in_maps


def kernel(hidden_states, position_ids, Wq, bq, Wk, bk, Wv, bv, Wo):
    in_maps = _host_inputs(hidden_states, position_ids,
                           Wq, bq, Wk, bk, Wv, bv, Wo)
    nc = _get_nc()
    last_err = None
    for attempt in range(3):
        try:
            res = bass_utils.run_bass_kernel_spmd(
                nc, in_maps, core_ids=list(range(N_CORES)))
            break
        except Exception as e:  # transient axon/device states clear on retry
            last_err = e
            import time
            time.sleep(15 * (attempt + 1))
    else:
        raise last_err
    out = np.concatenate([res.results[c]["y"] for c in range(N_CORES)], axis=0)
    return out.reshape(B, S, D)
